# revision 10
# baseline (speedup 1.0000x reference)
"""Trainium2 Bass kernel for nn_CompressiveMemory.

Math (B=128, H=64, D=64, N=M=D; see reference):
  binding stats:  mean_binding[m] = mean_{b,h,n} sum_d k[b,h,n,d] v[b,h,m,d]
                  (factored as ksum[b,h,d] = sum_n k[b,h,n,d];
                   bindsum[b,h,m] = sum_d v[b,h,m,d] ksum[b,h,d])
  mean_value[n,d]   = mean_{b,h} v[b,h,n,d]
  mean_key_norm[n]  = || mean_{b,h} k[b,h,n,d] ||_d
  scores[b,h,m,d]   = (sum_n q16[b,h,n,d] mk16[h,n,m]) / (memory_norm[d]+1e-6)
  memout[b,h,n,d]   = sum_m mv[h,n,m] scores[b,h,m,d]

Sharding: data-parallel over batch b across 8 cores (16 batches each);
per-head memory state replicated; batch-mean statistics partial-summed
on device and combined on host.

Host prep: q/k/v are pre-cast to fp16 (the reference itself computes the
q/k/v einsums in fp16) and pre-transposed to [H, N, B_shard, D] so every
DMA descriptor is a fully contiguous >=2KB run.

Per-core device kernel: loop over 32 head-pairs (hp). Each hp works on
[128, 1024] tiles: partitions = (head-in-pair, n), free = (batch, d).
PE does scores/memout matmuls with block-diagonal fp16 weights, fp16
column-sum matmuls (ksum), fp16 transposes of v, PSUM-accumulating
identity matmuls for the k/v batch sums, and one PSUM-accumulating
matmul chain for the binding partial sums.
"""

import numpy as np

B, H, D = 128, 64, 64
NCORES = 8
BS = B // NCORES  # 16 batches per core
NHP = H // 2  # 32 head pairs

_NC_CACHE = {}


def build_nc(n_hp=NHP, bs=BS):
    import concourse.bacc as bacc
    import concourse.tile as tile
    from concourse import mybir
    import concourse.bass as bass

    f32 = mybir.dt.float32
    f16 = mybir.dt.float16
    PSUM = bass.MemorySpace.PSUM

    nc = bacc.Bacc("TRN2", target_bir_lowering=False, debug=False,
                   num_devices=NCORES)

    qkv_d = nc.dram_tensor("qkv", [H, D, 3, bs, D], f16, kind="ExternalInput")
    mkbd_d = nc.dram_tensor("mk_bd", [128, NHP, 128], f16, kind="ExternalInput")
    mvtbd_d = nc.dram_tensor("mvT_bd", [128, NHP, 128], f16, kind="ExternalInput")
    inv_d = nc.dram_tensor("inv_big", [128, 512], f32, kind="ExternalInput")
    ones_d = nc.dram_tensor("ones_bd", [128, 2], f16, kind="ExternalInput")
    id_d = nc.dram_tensor("ident16", [128, 128], f16, kind="ExternalInput")

    out_d = nc.dram_tensor("out2", [H, D, 2, bs, D], f16, kind="ExternalOutput")
    bind_o = nc.dram_tensor("bind_o", [128, 2], f32, kind="ExternalOutput")
    acck_o = nc.dram_tensor("acck_o", [128, 512], f32, kind="ExternalOutput")
    accv_o = nc.dram_tensor("accv_o", [128, 512], f32, kind="ExternalOutput")

    nslab = bs // 8  # [128, 512] slabs per hp (8 batches each)

    with tile.TileContext(nc) as tc:
        with (
            tc.tile_pool(name="const", bufs=1) as cpool,
            tc.tile_pool(name="io", bufs=3) as iop,
            tc.tile_pool(name="sc", bufs=2 * nslab) as scp,
            tc.tile_pool(name="vt", bufs=4) as vtp,
            tc.tile_pool(name="ps_mm", bufs=2, space=PSUM) as ps_mm,
            tc.tile_pool(name="ps_vt", bufs=2, space=PSUM) as ps_vt,
            tc.tile_pool(name="ps_ks", bufs=1, space=PSUM) as ps_ks,
            tc.tile_pool(name="ps_acc", bufs=1, space=PSUM) as ps_acc,
        ):
            wq = cpool.tile([128, n_hp * 128], f16)
            nc.sync.dma_start(wq[:], mkbd_d[:, 0:n_hp])
            wv = cpool.tile([128, n_hp * 128], f16)
            nc.sync.dma_start(wv[:], mvtbd_d[:, 0:n_hp])
            inv_t = cpool.tile([128, 512], f32)
            nc.sync.dma_start(inv_t[:], inv_d[:])
            ones_t = cpool.tile([128, 2], f16)
            nc.sync.dma_start(ones_t[:], ones_d[:])
            id_t = cpool.tile([128, 128], f16)
            nc.sync.dma_start(id_t[:], id_d[:])

            bind_ps = ps_acc.tile([128, 2], f32, tag="bind")
            acck_ps = ps_acc.tile([128, 512], f32, tag="acck")
            accv_ps = ps_acc.tile([128, 512], f32, tag="accv")

            for hp in range(n_hp):
                first = hp == 0
                last = hp == n_hp - 1

                qkv16 = iop.tile([128, 3 * bs * D], f16, tag="qkv")
                nc.sync.dma_start(
                    qkv16[:],
                    qkv_d[2 * hp:2 * hp + 2].rearrange(
                        "hh n t b d -> (hh n) t b d"),
                )
                q16 = qkv16[:, 0:bs * D]
                k16 = qkv16[:, bs * D:2 * bs * D]
                v16 = qkv16[:, 2 * bs * D:3 * bs * D]

                # --- scores: psum[h'm, d] = mk_bd[hp].T @ q16 per batch ---
                out_tiles = []
                for s in range(nslab):
                    ps = ps_mm.tile([128, 512], f32, tag="mm")
                    nc.tensor.matmul(
                        ps[:],
                        wq[:, hp * 128:(hp + 1) * 128],
                        q16[:, s * 512:(s + 1) * 512],
                    )
                    ot = scp.tile([128, 1024], f16, tag="out")
                    nc.vector.tensor_mul(ot[:, 0:512], ps[:], inv_t[:])
                    out_tiles.append(ot)

                # --- stats: ksum via ones matmul, v transpose, accumulators ---
                ks_ps = ps_ks.tile([128, bs], f32)
                for p8 in range(bs // 2):
                    nc.tensor.matmul(
                        ks_ps[:, 2 * p8:2 * p8 + 2],
                        k16[:, p8 * 128:(p8 + 1) * 128],
                        ones_t[:],
                    )
                ks16 = iop.tile([128, bs], f16, tag="ks16")
                nc.scalar.copy(ks16[:], ks_ps[:])

                for s in range(nslab):
                    nc.tensor.matmul(
                        acck_ps[:],
                        id_t[:],
                        k16[:, s * 512:(s + 1) * 512],
                        start=(first and s == 0),
                        stop=(last and s == nslab - 1),
                        skip_group_check=True,
                    )
                    nc.tensor.matmul(
                        accv_ps[:],
                        id_t[:],
                        v16[:, s * 512:(s + 1) * 512],
                        start=(first and s == 0),
                        stop=(last and s == nslab - 1),
                        skip_group_check=True,
                    )

                vt_ps = ps_vt.tile([128, (bs // 2) * 128], f16, tag="vtps")
                for p8 in range(bs // 2):
                    nc.tensor.transpose(
                        vt_ps[:, p8 * 128:(p8 + 1) * 128],
                        v16[:, p8 * 128:(p8 + 1) * 128],
                        id_t[:],
                    )
                vt16 = vtp.tile([128, (bs // 2) * 128], f16, tag="vt16")
                nc.vector.tensor_copy(vt16[:], vt_ps[:])
                for p8 in range(bs // 2):
                    nc.tensor.matmul(
                        bind_ps[:],
                        vt16[:, p8 * 128:(p8 + 1) * 128],
                        ks16[:, 2 * p8:2 * p8 + 2],
                        start=(first and p8 == 0),
                        stop=(last and p8 == bs // 2 - 1),
                        skip_group_check=True,
                    )

                # --- memout: psum[h'n, d] = mvT_bd[hp].T @ sc16 per batch ---
                for s in range(nslab):
                    ps2 = ps_mm.tile([128, 512], f32, tag="mm")
                    ot = out_tiles[s]
                    nc.tensor.matmul(
                        ps2[:],
                        wv[:, hp * 128:(hp + 1) * 128],
                        ot[:, 0:512],
                    )
                    if s % 2 == 0:
                        nc.vector.tensor_copy(ot[:, 512:1024], ps2[:])
                        nc.gpsimd.dma_start(
                            out_d[2 * hp:2 * hp + 2, :, :, s * 8:(s + 1) * 8]
                            .rearrange("hh x t b d -> (hh x) t b d"),
                            ot[:],
                        )
                    else:
                        nc.scalar.copy(ot[:, 512:1024], ps2[:])
                        nc.scalar.dma_start(
                            out_d[2 * hp:2 * hp + 2, :, :, s * 8:(s + 1) * 8]
                            .rearrange("hh x t b d -> (hh x) t b d"),
                            ot[:],
                        )

            bind_sb = cpool.tile([128, 2], f32)
            nc.vector.tensor_copy(bind_sb[:], bind_ps[:])
            nc.sync.dma_start(bind_o[:], bind_sb[:])
            acck_sb = cpool.tile([128, 512], f32)
            nc.vector.tensor_copy(acck_sb[:], acck_ps[:])
            nc.sync.dma_start(acck_o[:], acck_sb[:])
            accv_sb = cpool.tile([128, 512], f32)
            nc.vector.tensor_copy(accv_sb[:], accv_ps[:])
            nc.sync.dma_start(accv_o[:], accv_sb[:])

    nc.compile()
    return nc


def get_nc():
    if "nc" not in _NC_CACHE:
        _NC_CACHE["nc"] = build_nc()
    return _NC_CACHE["nc"]


def host_aux(memory_key, memory_norm, memory_value):
    mk16 = memory_key.astype(np.float16)
    mk_bd = np.zeros((NHP, 128, 128), np.float16)
    mvT_bd = np.zeros((NHP, 128, 128), np.float16)
    for hp in range(NHP):
        mk_bd[hp, :64, :64] = mk16[2 * hp]
        mk_bd[hp, 64:, 64:] = mk16[2 * hp + 1]
        mvT_bd[hp, :64, :64] = memory_value[2 * hp].T.astype(np.float16)
        mvT_bd[hp, 64:, 64:] = memory_value[2 * hp + 1].T.astype(np.float16)
    inv = (1.0 / (memory_norm.astype(np.float64) + 1e-6)).astype(np.float32)
    inv_big = np.tile(inv, (128, 8))
    ones_bd = np.zeros((128, 2), np.float16)
    ones_bd[:64, 0] = 1.0
    ones_bd[64:, 1] = 1.0
    ident16 = np.eye(128, dtype=np.float16)
    return {
        "mk_bd": np.ascontiguousarray(mk_bd.transpose(1, 0, 2)),
        "mvT_bd": np.ascontiguousarray(mvT_bd.transpose(1, 0, 2)),
        "inv_big": np.ascontiguousarray(inv_big, np.float32),
        "ones_bd": ones_bd,
        "ident16": ident16,
    }


def finalize_stats(bind, acck, accv, memory_norm, compression_rate):
    """bind/acck/accv: summed device partials across cores."""
    bind_total = bind[:64, 0] + bind[64:, 1]  # [64] indexed by m
    mean_binding = bind_total / float(B * H * D)
    acck = acck.reshape(128, 8, 64).sum(axis=1)
    accv = accv.reshape(128, 8, 64).sum(axis=1)
    mean_key = (acck[:64] + acck[64:]) / float(B * H)
    mean_value = (accv[:64] + accv[64:]) / float(B * H)
    mean_key_norm = np.linalg.norm(mean_key, axis=-1)
    new_memory_key = mean_binding[:, None] * mean_value
    new_memory_norm = memory_norm + mean_key_norm
    if new_memory_norm.mean() > 0.9:
        factor = compression_rate
    else:
        factor = np.ones_like(compression_rate)
    new_memory_key = (new_memory_key * factor[:, None]).astype(np.float32)
    new_memory_norm = (new_memory_norm * factor).astype(np.float32)
    return new_memory_key, new_memory_norm


def kernel(query, key, value, memory_key, memory_norm, memory_value,
           compression_rate):
    from concourse.bass_utils import run_bass_kernel_spmd

    query = np.asarray(query, np.float32)
    key = np.asarray(key, np.float32)
    value = np.asarray(value, np.float32)
    memory_key = np.asarray(memory_key, np.float32)
    memory_norm = np.asarray(memory_norm, np.float32)
    memory_value = np.asarray(memory_value, np.float32)
    compression_rate = np.asarray(compression_rate, np.float32)

    nc = get_nc()
    aux = host_aux(memory_key, memory_norm, memory_value)
    qkv = np.stack([query, key, value]).transpose(2, 3, 0, 1, 4)
    qkv16 = qkv.astype(np.float16)  # [H, N, 3, B, D]
    in_maps = []
    for c in range(NCORES):
        sl = slice(c * BS, (c + 1) * BS)
        in_maps.append({
            "qkv": np.ascontiguousarray(qkv16[:, :, :, sl]),
            **aux,
        })
    res = run_bass_kernel_spmd(nc, in_maps, list(range(NCORES)))
    results = res.results

    out2 = np.concatenate([results[c]["out2"] for c in range(NCORES)], axis=3)
    scores = np.ascontiguousarray(
        out2[:, :, 0].transpose(2, 0, 1, 3)).astype(np.float32)
    memout = np.ascontiguousarray(
        out2[:, :, 1].transpose(2, 0, 1, 3)).astype(np.float32)
    bind = np.sum([results[c]["bind_o"] for c in range(NCORES)], axis=0,
                  dtype=np.float64).astype(np.float32)
    acck = np.sum([results[c]["acck_o"] for c in range(NCORES)], axis=0,
                  dtype=np.float64).astype(np.float32)
    accv = np.sum([results[c]["accv_o"] for c in range(NCORES)], axis=0,
                  dtype=np.float64).astype(np.float32)

    new_memory_key, new_memory_norm = finalize_stats(
        bind, acck, accv, memory_norm, compression_rate)
    return memout, scores, new_memory_key, new_memory_norm


# revision 12
# speedup vs baseline: 1.0164x; 1.0164x over previous
"""Trainium2 Bass kernel for nn_CompressiveMemory.

Math (B=128, H=64, D=64, N=M=D; see reference):
  binding stats:  mean_binding[m] = mean_{b,h,n} sum_d k[b,h,n,d] v[b,h,m,d]
                  (factored as ksum[b,h,d] = sum_n k[b,h,n,d];
                   bindsum[b,h,m] = sum_d v[b,h,m,d] ksum[b,h,d])
  mean_value[n,d]   = mean_{b,h} v[b,h,n,d]
  mean_key_norm[n]  = || mean_{b,h} k[b,h,n,d] ||_d
  scores[b,h,m,d]   = (sum_n q16[b,h,n,d] mk16[h,n,m]) / (memory_norm[d]+1e-6)
  memout[b,h,n,d]   = sum_m mv[h,n,m] scores[b,h,m,d]

Sharding: data-parallel over batch b across 8 cores (16 batches each);
per-head memory state replicated; batch-mean statistics partial-summed
on device and combined on host.

Host prep: q/k/v are pre-cast to fp16 (the reference itself computes the
q/k/v einsums in fp16) and pre-transposed to [H, N, B_shard, D] so every
DMA descriptor is a fully contiguous >=2KB run.

Per-core device kernel: loop over 32 head-pairs (hp). Each hp works on
[128, 1024] tiles: partitions = (head-in-pair, n), free = (batch, d).
PE does scores/memout matmuls with block-diagonal fp16 weights, fp16
column-sum matmuls (ksum), fp16 transposes of v, PSUM-accumulating
identity matmuls for the k/v batch sums, and one PSUM-accumulating
matmul chain for the binding partial sums.
"""

import numpy as np

B, H, D = 128, 64, 64
NCORES = 8
BS = B // NCORES  # 16 batches per core
NHP = H // 2  # 32 head pairs

_NC_CACHE = {}


def build_nc(n_hp=NHP, bs=BS):
    import concourse.bacc as bacc
    import concourse.tile as tile
    from concourse import mybir
    import concourse.bass as bass

    f32 = mybir.dt.float32
    f16 = mybir.dt.float16
    PSUM = bass.MemorySpace.PSUM

    nc = bacc.Bacc("TRN2", target_bir_lowering=False, debug=False,
                   num_devices=NCORES)

    qkv_d = nc.dram_tensor("qkv", [H, D, 3, bs, D], f16, kind="ExternalInput")
    mkbd_d = nc.dram_tensor("mk_bd", [128, NHP, 128], f16, kind="ExternalInput")
    mvtbd_d = nc.dram_tensor("mvT_bd", [128, NHP, 128], f16, kind="ExternalInput")
    inv_d = nc.dram_tensor("inv_big", [128, 512], f32, kind="ExternalInput")
    ones_d = nc.dram_tensor("ones_bd", [128, 2], f16, kind="ExternalInput")
    id_d = nc.dram_tensor("ident16", [128, 128], f16, kind="ExternalInput")

    out_d = nc.dram_tensor("out2", [H, D, 2, bs, D], f16, kind="ExternalOutput")
    bind_o = nc.dram_tensor("bind_o", [128, 2], f32, kind="ExternalOutput")
    acck_o = nc.dram_tensor("acck_o", [128, 512], f32, kind="ExternalOutput")
    accv_o = nc.dram_tensor("accv_o", [128, 512], f32, kind="ExternalOutput")

    nslab = bs // 8  # [128, 512] slabs per hp (8 batches each)

    with tile.TileContext(nc) as tc:
        with (
            tc.tile_pool(name="const", bufs=1) as cpool,
            tc.tile_pool(name="io", bufs=3) as iop,
            tc.tile_pool(name="sc", bufs=2 * nslab) as scp,
            tc.tile_pool(name="vt", bufs=4) as vtp,
            tc.tile_pool(name="ps_mm", bufs=3, space=PSUM) as ps_mm,
            tc.tile_pool(name="ps_vt", bufs=1, space=PSUM) as ps_vt,
            tc.tile_pool(name="ps_ks", bufs=1, space=PSUM) as ps_ks,
            tc.tile_pool(name="ps_acc", bufs=1, space=PSUM) as ps_acc,
        ):
            qkv_pre = iop.tile([128, 3 * bs * D], f16, tag="qkv")
            nc.sync.dma_start(
                qkv_pre[:],
                qkv_d[0:2].rearrange("hh n t b d -> (hh n) t b d"),
            )
            wq = cpool.tile([128, n_hp * 128], f16)
            nc.sync.dma_start(wq[:], mkbd_d[:, 0:n_hp])
            wv = cpool.tile([128, n_hp * 128], f16)
            nc.sync.dma_start(wv[:], mvtbd_d[:, 0:n_hp])
            inv_t = cpool.tile([128, 512], f32)
            nc.sync.dma_start(inv_t[:], inv_d[:])
            ones_t = cpool.tile([128, 2], f16)
            nc.sync.dma_start(ones_t[:], ones_d[:])
            id_t = cpool.tile([128, 128], f16)
            nc.sync.dma_start(id_t[:], id_d[:])

            bind_ps = ps_acc.tile([128, 2], f32, tag="bind")
            acck_ps = ps_acc.tile([128, 512], f32, tag="acck")
            accv_ps = ps_acc.tile([128, 512], f32, tag="accv")

            for hp in range(n_hp):
                first = hp == 0
                last = hp == n_hp - 1

                if hp == 0:
                    qkv16 = qkv_pre
                else:
                    qkv16 = iop.tile([128, 3 * bs * D], f16, tag="qkv")
                    nc.sync.dma_start(
                        qkv16[:],
                        qkv_d[2 * hp:2 * hp + 2].rearrange(
                            "hh n t b d -> (hh n) t b d"),
                    )
                q16 = qkv16[:, 0:bs * D]
                k16 = qkv16[:, bs * D:2 * bs * D]
                v16 = qkv16[:, 2 * bs * D:3 * bs * D]

                # --- scores: psum[h'm, d] = mk_bd[hp].T @ q16 per batch ---
                out_tiles = []
                for s in range(nslab):
                    ps = ps_mm.tile([128, 512], f32, tag="mm")
                    nc.tensor.matmul(
                        ps[:],
                        wq[:, hp * 128:(hp + 1) * 128],
                        q16[:, s * 512:(s + 1) * 512],
                    )
                    ot = scp.tile([128, 1024], f16, tag="out")
                    nc.vector.tensor_mul(ot[:, 0:512], ps[:], inv_t[:])
                    out_tiles.append(ot)

                # --- stats: ksum via ones matmul, v transpose, accumulators ---
                ks_ps = ps_ks.tile([128, bs], f32)
                for p8 in range(bs // 2):
                    nc.tensor.matmul(
                        ks_ps[:, 2 * p8:2 * p8 + 2],
                        k16[:, p8 * 128:(p8 + 1) * 128],
                        ones_t[:],
                    )
                ks16 = iop.tile([128, bs], f16, tag="ks16")
                nc.scalar.copy(ks16[:], ks_ps[:])

                for s in range(nslab):
                    nc.tensor.matmul(
                        acck_ps[:],
                        id_t[:],
                        k16[:, s * 512:(s + 1) * 512],
                        start=(first and s == 0),
                        stop=(last and s == nslab - 1),
                        skip_group_check=True,
                    )
                    nc.tensor.matmul(
                        accv_ps[:],
                        id_t[:],
                        v16[:, s * 512:(s + 1) * 512],
                        start=(first and s == 0),
                        stop=(last and s == nslab - 1),
                        skip_group_check=True,
                    )

                vt_ps = ps_vt.tile([128, (bs // 2) * 128], f16, tag="vtps")
                for p8 in range(bs // 2):
                    nc.tensor.transpose(
                        vt_ps[:, p8 * 128:(p8 + 1) * 128],
                        v16[:, p8 * 128:(p8 + 1) * 128],
                        id_t[:],
                    )
                vt16 = vtp.tile([128, (bs // 2) * 128], f16, tag="vt16")
                nc.vector.tensor_copy(vt16[:], vt_ps[:])
                for p8 in range(bs // 2):
                    nc.tensor.matmul(
                        bind_ps[:],
                        vt16[:, p8 * 128:(p8 + 1) * 128],
                        ks16[:, 2 * p8:2 * p8 + 2],
                        start=(first and p8 == 0),
                        stop=(last and p8 == bs // 2 - 1),
                        skip_group_check=True,
                    )

                # --- memout: psum[h'n, d] = mvT_bd[hp].T @ sc16 per batch ---
                for s in range(nslab):
                    ps2 = ps_mm.tile([128, 512], f32, tag="mm")
                    ot = out_tiles[s]
                    nc.tensor.matmul(
                        ps2[:],
                        wv[:, hp * 128:(hp + 1) * 128],
                        ot[:, 0:512],
                    )
                    if s % 2 == 0:
                        nc.vector.tensor_copy(ot[:, 512:1024], ps2[:])
                        nc.gpsimd.dma_start(
                            out_d[2 * hp:2 * hp + 2, :, :, s * 8:(s + 1) * 8]
                            .rearrange("hh x t b d -> (hh x) t b d"),
                            ot[:],
                        )
                    else:
                        nc.scalar.copy(ot[:, 512:1024], ps2[:])
                        nc.scalar.dma_start(
                            out_d[2 * hp:2 * hp + 2, :, :, s * 8:(s + 1) * 8]
                            .rearrange("hh x t b d -> (hh x) t b d"),
                            ot[:],
                        )

            bind_sb = cpool.tile([128, 2], f32)
            nc.vector.tensor_copy(bind_sb[:], bind_ps[:])
            nc.sync.dma_start(bind_o[:], bind_sb[:])
            acck_sb = cpool.tile([128, 512], f32)
            nc.vector.tensor_copy(acck_sb[:], acck_ps[:])
            nc.sync.dma_start(acck_o[:], acck_sb[:])
            accv_sb = cpool.tile([128, 512], f32)
            nc.vector.tensor_copy(accv_sb[:], accv_ps[:])
            nc.sync.dma_start(accv_o[:], accv_sb[:])

    nc.compile()
    return nc


def get_nc():
    if "nc" not in _NC_CACHE:
        _NC_CACHE["nc"] = build_nc()
    return _NC_CACHE["nc"]


def host_aux(memory_key, memory_norm, memory_value):
    mk16 = memory_key.astype(np.float16)
    mk_bd = np.zeros((NHP, 128, 128), np.float16)
    mvT_bd = np.zeros((NHP, 128, 128), np.float16)
    for hp in range(NHP):
        mk_bd[hp, :64, :64] = mk16[2 * hp]
        mk_bd[hp, 64:, 64:] = mk16[2 * hp + 1]
        mvT_bd[hp, :64, :64] = memory_value[2 * hp].T.astype(np.float16)
        mvT_bd[hp, 64:, 64:] = memory_value[2 * hp + 1].T.astype(np.float16)
    inv = (1.0 / (memory_norm.astype(np.float64) + 1e-6)).astype(np.float32)
    inv_big = np.tile(inv, (128, 8))
    ones_bd = np.zeros((128, 2), np.float16)
    ones_bd[:64, 0] = 1.0
    ones_bd[64:, 1] = 1.0
    ident16 = np.eye(128, dtype=np.float16)
    return {
        "mk_bd": np.ascontiguousarray(mk_bd.transpose(1, 0, 2)),
        "mvT_bd": np.ascontiguousarray(mvT_bd.transpose(1, 0, 2)),
        "inv_big": np.ascontiguousarray(inv_big, np.float32),
        "ones_bd": ones_bd,
        "ident16": ident16,
    }


def finalize_stats(bind, acck, accv, memory_norm, compression_rate):
    """bind/acck/accv: summed device partials across cores."""
    bind_total = bind[:64, 0] + bind[64:, 1]  # [64] indexed by m
    mean_binding = bind_total / float(B * H * D)
    acck = acck.reshape(128, 8, 64).sum(axis=1)
    accv = accv.reshape(128, 8, 64).sum(axis=1)
    mean_key = (acck[:64] + acck[64:]) / float(B * H)
    mean_value = (accv[:64] + accv[64:]) / float(B * H)
    mean_key_norm = np.linalg.norm(mean_key, axis=-1)
    new_memory_key = mean_binding[:, None] * mean_value
    new_memory_norm = memory_norm + mean_key_norm
    if new_memory_norm.mean() > 0.9:
        factor = compression_rate
    else:
        factor = np.ones_like(compression_rate)
    new_memory_key = (new_memory_key * factor[:, None]).astype(np.float32)
    new_memory_norm = (new_memory_norm * factor).astype(np.float32)
    return new_memory_key, new_memory_norm


def kernel(query, key, value, memory_key, memory_norm, memory_value,
           compression_rate):
    from concourse.bass_utils import run_bass_kernel_spmd

    query = np.asarray(query, np.float32)
    key = np.asarray(key, np.float32)
    value = np.asarray(value, np.float32)
    memory_key = np.asarray(memory_key, np.float32)
    memory_norm = np.asarray(memory_norm, np.float32)
    memory_value = np.asarray(memory_value, np.float32)
    compression_rate = np.asarray(compression_rate, np.float32)

    nc = get_nc()
    aux = host_aux(memory_key, memory_norm, memory_value)
    qkv = np.stack([query, key, value]).transpose(2, 3, 0, 1, 4)
    qkv16 = qkv.astype(np.float16)  # [H, N, 3, B, D]
    in_maps = []
    for c in range(NCORES):
        sl = slice(c * BS, (c + 1) * BS)
        in_maps.append({
            "qkv": np.ascontiguousarray(qkv16[:, :, :, sl]),
            **aux,
        })
    res = run_bass_kernel_spmd(nc, in_maps, list(range(NCORES)))
    results = res.results

    out2 = np.concatenate([results[c]["out2"] for c in range(NCORES)], axis=3)
    scores = np.ascontiguousarray(
        out2[:, :, 0].transpose(2, 0, 1, 3)).astype(np.float32)
    memout = np.ascontiguousarray(
        out2[:, :, 1].transpose(2, 0, 1, 3)).astype(np.float32)
    bind = np.sum([results[c]["bind_o"] for c in range(NCORES)], axis=0,
                  dtype=np.float64).astype(np.float32)
    acck = np.sum([results[c]["acck_o"] for c in range(NCORES)], axis=0,
                  dtype=np.float64).astype(np.float32)
    accv = np.sum([results[c]["accv_o"] for c in range(NCORES)], axis=0,
                  dtype=np.float64).astype(np.float32)

    new_memory_key, new_memory_norm = finalize_stats(
        bind, acck, accv, memory_norm, compression_rate)
    return memout, scores, new_memory_key, new_memory_norm


# revision 13
# speedup vs baseline: 1.0956x; 1.0779x over previous
"""Trainium2 Bass kernel for nn_CompressiveMemory.

Math (B=128, H=64, D=64, N=M=D; see reference):
  binding stats:  mean_binding[m] = mean_{b,h,n} sum_d k[b,h,n,d] v[b,h,m,d]
                  (factored as ksum[b,h,d] = sum_n k[b,h,n,d];
                   bindsum[b,h,m] = sum_d v[b,h,m,d] ksum[b,h,d])
  mean_value[n,d]   = mean_{b,h} v[b,h,n,d]
  mean_key_norm[n]  = || mean_{b,h} k[b,h,n,d] ||_d
  scores[b,h,m,d]   = (sum_n q16[b,h,n,d] mk16[h,n,m]) / (memory_norm[d]+1e-6)
  memout[b,h,n,d]   = sum_m mv[h,n,m] scores[b,h,m,d]

Sharding: data-parallel over batch b across 8 cores (16 batches each);
per-head memory state replicated; batch-mean statistics partial-summed
on device and combined on host.

Host prep: q/k/v are pre-cast to fp16 (the reference itself computes the
q/k/v einsums in fp16) and pre-transposed to [H, N, B_shard, D] so every
DMA descriptor is a fully contiguous >=2KB run.

Per-core device kernel: loop over 32 head-pairs (hp). Each hp works on
[128, 1024] tiles: partitions = (head-in-pair, n), free = (batch, d).
PE does scores/memout matmuls with block-diagonal fp16 weights, fp16
column-sum matmuls (ksum), fp16 transposes of v, PSUM-accumulating
identity matmuls for the k/v batch sums, and one PSUM-accumulating
matmul chain for the binding partial sums.
"""

import numpy as np

B, H, D = 128, 64, 64
NCORES = 8
BS = B // NCORES  # 16 batches per core
NHP = H // 2  # 32 head pairs

_NC_CACHE = {}


def build_nc(n_hp=NHP, bs=BS):
    import concourse.bacc as bacc
    import concourse.tile as tile
    from concourse import mybir
    import concourse.bass as bass

    f32 = mybir.dt.float32
    f16 = mybir.dt.float16
    PSUM = bass.MemorySpace.PSUM

    nc = bacc.Bacc("TRN2", target_bir_lowering=False, debug=False,
                   num_devices=NCORES)

    qkv_d = nc.dram_tensor("qkv", [H, D, 3, bs, D], f16, kind="ExternalInput")
    mkbd_d = nc.dram_tensor("mk_bd", [128, NHP, 128], f16, kind="ExternalInput")
    mvtbd_d = nc.dram_tensor("mvT_bd", [128, NHP, 128], f16, kind="ExternalInput")
    inv_d = nc.dram_tensor("inv_big", [128, 512], f32, kind="ExternalInput")
    ones_d = nc.dram_tensor("ones_bd", [128, 2], f16, kind="ExternalInput")
    id_d = nc.dram_tensor("ident16", [128, 128], f16, kind="ExternalInput")

    out_d = nc.dram_tensor("out2", [H, D, 2, bs, D], f16, kind="ExternalOutput")
    bind_o = nc.dram_tensor("bind_o", [128, 2], f32, kind="ExternalOutput")
    acck_o = nc.dram_tensor("acck_o", [128, 512], f32, kind="ExternalOutput")
    accv_o = nc.dram_tensor("accv_o", [128, 512], f32, kind="ExternalOutput")

    nslab = bs // 8  # [128, 512] slabs per hp (8 batches each)

    with tile.TileContext(nc) as tc:
        with (
            tc.tile_pool(name="const", bufs=1) as cpool,
            tc.tile_pool(name="io", bufs=4) as iop,
            tc.tile_pool(name="sc", bufs=3 * nslab) as scp,
            tc.tile_pool(name="vt", bufs=4) as vtp,
            tc.tile_pool(name="ps_mm", bufs=3, space=PSUM) as ps_mm,
            tc.tile_pool(name="ps_vt", bufs=1, space=PSUM) as ps_vt,
            tc.tile_pool(name="ps_ks", bufs=1, space=PSUM) as ps_ks,
            tc.tile_pool(name="ps_acc", bufs=1, space=PSUM) as ps_acc,
        ):
            qkv_pre = iop.tile([128, 3 * bs * D], f16, tag="qkv")
            nc.sync.dma_start(
                qkv_pre[:],
                qkv_d[0:2].rearrange("hh n t b d -> (hh n) t b d"),
            )
            wq = cpool.tile([128, n_hp * 128], f16)
            nc.sync.dma_start(wq[:], mkbd_d[:, 0:n_hp])
            wv = cpool.tile([128, n_hp * 128], f16)
            nc.sync.dma_start(wv[:], mvtbd_d[:, 0:n_hp])
            inv_t = cpool.tile([128, 512], f32)
            nc.sync.dma_start(inv_t[:], inv_d[:])
            ones_t = cpool.tile([128, 2], f16)
            nc.sync.dma_start(ones_t[:], ones_d[:])
            id_t = cpool.tile([128, 128], f16)
            nc.sync.dma_start(id_t[:], id_d[:])

            bind_ps = ps_acc.tile([128, 2], f32, tag="bind")
            acck_ps = ps_acc.tile([128, 512], f32, tag="acck")
            accv_ps = ps_acc.tile([128, 512], f32, tag="accv")

            for hp in range(n_hp):
                first = hp == 0
                last = hp == n_hp - 1

                if hp == 0:
                    qkv16 = qkv_pre
                else:
                    qkv16 = iop.tile([128, 3 * bs * D], f16, tag="qkv")
                    nc.sync.dma_start(
                        qkv16[:],
                        qkv_d[2 * hp:2 * hp + 2].rearrange(
                            "hh n t b d -> (hh n) t b d"),
                    )
                q16 = qkv16[:, 0:bs * D]
                k16 = qkv16[:, bs * D:2 * bs * D]
                v16 = qkv16[:, 2 * bs * D:3 * bs * D]

                # --- scores: psum[h'm, d] = mk_bd[hp].T @ q16 per batch ---
                out_tiles = []
                for s in range(nslab):
                    ps = ps_mm.tile([128, 512], f32, tag="mm")
                    nc.tensor.matmul(
                        ps[:],
                        wq[:, hp * 128:(hp + 1) * 128],
                        q16[:, s * 512:(s + 1) * 512],
                    )
                    ot = scp.tile([128, 1024], f16, tag="out")
                    nc.vector.tensor_mul(ot[:, 0:512], ps[:], inv_t[:])
                    out_tiles.append(ot)

                # --- stats: ksum via ones matmul, v transpose, accumulators ---
                ks_ps = ps_ks.tile([128, bs], f32)
                for p8 in range(bs // 2):
                    nc.tensor.matmul(
                        ks_ps[:, 2 * p8:2 * p8 + 2],
                        k16[:, p8 * 128:(p8 + 1) * 128],
                        ones_t[:],
                    )
                ks16 = iop.tile([128, bs], f16, tag="ks16")
                nc.scalar.copy(ks16[:], ks_ps[:])

                for s in range(nslab):
                    nc.tensor.matmul(
                        acck_ps[:],
                        id_t[:],
                        k16[:, s * 512:(s + 1) * 512],
                        start=(first and s == 0),
                        stop=(last and s == nslab - 1),
                        skip_group_check=True,
                    )
                    nc.tensor.matmul(
                        accv_ps[:],
                        id_t[:],
                        v16[:, s * 512:(s + 1) * 512],
                        start=(first and s == 0),
                        stop=(last and s == nslab - 1),
                        skip_group_check=True,
                    )

                vt_ps = ps_vt.tile([128, (bs // 2) * 128], f16, tag="vtps")
                for p8 in range(bs // 2):
                    nc.tensor.transpose(
                        vt_ps[:, p8 * 128:(p8 + 1) * 128],
                        v16[:, p8 * 128:(p8 + 1) * 128],
                        id_t[:],
                    )
                vt16 = vtp.tile([128, (bs // 2) * 128], f16, tag="vt16")
                nc.vector.tensor_copy(vt16[:], vt_ps[:])
                for p8 in range(bs // 2):
                    nc.tensor.matmul(
                        bind_ps[:],
                        vt16[:, p8 * 128:(p8 + 1) * 128],
                        ks16[:, 2 * p8:2 * p8 + 2],
                        start=(first and p8 == 0),
                        stop=(last and p8 == bs // 2 - 1),
                        skip_group_check=True,
                    )

                # --- memout: psum[h'n, d] = mvT_bd[hp].T @ sc16 per batch ---
                for s in range(nslab):
                    ps2 = ps_mm.tile([128, 512], f32, tag="mm")
                    ot = out_tiles[s]
                    nc.tensor.matmul(
                        ps2[:],
                        wv[:, hp * 128:(hp + 1) * 128],
                        ot[:, 0:512],
                    )
                    if s % 2 == 0:
                        nc.vector.tensor_copy(ot[:, 512:1024], ps2[:])
                        nc.gpsimd.dma_start(
                            out_d[2 * hp:2 * hp + 2, :, :, s * 8:(s + 1) * 8]
                            .rearrange("hh x t b d -> (hh x) t b d"),
                            ot[:],
                        )
                    else:
                        nc.scalar.copy(ot[:, 512:1024], ps2[:])
                        nc.scalar.dma_start(
                            out_d[2 * hp:2 * hp + 2, :, :, s * 8:(s + 1) * 8]
                            .rearrange("hh x t b d -> (hh x) t b d"),
                            ot[:],
                        )

            bind_sb = cpool.tile([128, 2], f32)
            nc.vector.tensor_copy(bind_sb[:], bind_ps[:])
            nc.sync.dma_start(bind_o[:], bind_sb[:])
            acck_sb = cpool.tile([128, 512], f32)
            nc.vector.tensor_copy(acck_sb[:], acck_ps[:])
            nc.sync.dma_start(acck_o[:], acck_sb[:])
            accv_sb = cpool.tile([128, 512], f32)
            nc.vector.tensor_copy(accv_sb[:], accv_ps[:])
            nc.sync.dma_start(accv_o[:], accv_sb[:])

    nc.compile()
    return nc


def get_nc():
    if "nc" not in _NC_CACHE:
        _NC_CACHE["nc"] = build_nc()
    return _NC_CACHE["nc"]


def host_aux(memory_key, memory_norm, memory_value):
    mk16 = memory_key.astype(np.float16)
    mk_bd = np.zeros((NHP, 128, 128), np.float16)
    mvT_bd = np.zeros((NHP, 128, 128), np.float16)
    for hp in range(NHP):
        mk_bd[hp, :64, :64] = mk16[2 * hp]
        mk_bd[hp, 64:, 64:] = mk16[2 * hp + 1]
        mvT_bd[hp, :64, :64] = memory_value[2 * hp].T.astype(np.float16)
        mvT_bd[hp, 64:, 64:] = memory_value[2 * hp + 1].T.astype(np.float16)
    inv = (1.0 / (memory_norm.astype(np.float64) + 1e-6)).astype(np.float32)
    inv_big = np.tile(inv, (128, 8))
    ones_bd = np.zeros((128, 2), np.float16)
    ones_bd[:64, 0] = 1.0
    ones_bd[64:, 1] = 1.0
    ident16 = np.eye(128, dtype=np.float16)
    return {
        "mk_bd": np.ascontiguousarray(mk_bd.transpose(1, 0, 2)),
        "mvT_bd": np.ascontiguousarray(mvT_bd.transpose(1, 0, 2)),
        "inv_big": np.ascontiguousarray(inv_big, np.float32),
        "ones_bd": ones_bd,
        "ident16": ident16,
    }


def finalize_stats(bind, acck, accv, memory_norm, compression_rate):
    """bind/acck/accv: summed device partials across cores."""
    bind_total = bind[:64, 0] + bind[64:, 1]  # [64] indexed by m
    mean_binding = bind_total / float(B * H * D)
    acck = acck.reshape(128, 8, 64).sum(axis=1)
    accv = accv.reshape(128, 8, 64).sum(axis=1)
    mean_key = (acck[:64] + acck[64:]) / float(B * H)
    mean_value = (accv[:64] + accv[64:]) / float(B * H)
    mean_key_norm = np.linalg.norm(mean_key, axis=-1)
    new_memory_key = mean_binding[:, None] * mean_value
    new_memory_norm = memory_norm + mean_key_norm
    if new_memory_norm.mean() > 0.9:
        factor = compression_rate
    else:
        factor = np.ones_like(compression_rate)
    new_memory_key = (new_memory_key * factor[:, None]).astype(np.float32)
    new_memory_norm = (new_memory_norm * factor).astype(np.float32)
    return new_memory_key, new_memory_norm


def kernel(query, key, value, memory_key, memory_norm, memory_value,
           compression_rate):
    from concourse.bass_utils import run_bass_kernel_spmd

    query = np.asarray(query, np.float32)
    key = np.asarray(key, np.float32)
    value = np.asarray(value, np.float32)
    memory_key = np.asarray(memory_key, np.float32)
    memory_norm = np.asarray(memory_norm, np.float32)
    memory_value = np.asarray(memory_value, np.float32)
    compression_rate = np.asarray(compression_rate, np.float32)

    nc = get_nc()
    aux = host_aux(memory_key, memory_norm, memory_value)
    qkv = np.stack([query, key, value]).transpose(2, 3, 0, 1, 4)
    qkv16 = qkv.astype(np.float16)  # [H, N, 3, B, D]
    in_maps = []
    for c in range(NCORES):
        sl = slice(c * BS, (c + 1) * BS)
        in_maps.append({
            "qkv": np.ascontiguousarray(qkv16[:, :, :, sl]),
            **aux,
        })
    res = run_bass_kernel_spmd(nc, in_maps, list(range(NCORES)))
    results = res.results

    out2 = np.concatenate([results[c]["out2"] for c in range(NCORES)], axis=3)
    scores = np.ascontiguousarray(
        out2[:, :, 0].transpose(2, 0, 1, 3)).astype(np.float32)
    memout = np.ascontiguousarray(
        out2[:, :, 1].transpose(2, 0, 1, 3)).astype(np.float32)
    bind = np.sum([results[c]["bind_o"] for c in range(NCORES)], axis=0,
                  dtype=np.float64).astype(np.float32)
    acck = np.sum([results[c]["acck_o"] for c in range(NCORES)], axis=0,
                  dtype=np.float64).astype(np.float32)
    accv = np.sum([results[c]["accv_o"] for c in range(NCORES)], axis=0,
                  dtype=np.float64).astype(np.float32)

    new_memory_key, new_memory_norm = finalize_stats(
        bind, acck, accv, memory_norm, compression_rate)
    return memout, scores, new_memory_key, new_memory_norm
